# revision 19
# baseline (speedup 1.0000x reference)
"""Trainium2 Bass kernel for nn_FilmLayerNetwork.

Sharding: one NeuronCore per SMAB processor (NPROC = 8 = n_cores).
Each core computes its processor's full 512-map output slice.

v2 performance notes (on top of the v1 notes in kernel_v1.py.bak):
- ALL matmuls run in bf16: fp32 matmuls execute as LOW/HIGH instruction
  pairs (2x instructions, no FWL weight loads), and the output error
  budget is enormous (trans feeds mix*regs with regs ~1e-3, output gate
  is 2e-2). Weights are packed bf16 host-side (halves DMA bytes too);
  PSUM->SBUF copies cast to bf16 on the fly.
- Input DMAs are issued from TWO engine queues (sync: pxb+w1,
  gpsimd: pxf+wqr+b97+b16+wa) so descriptor issue (~0.6us each) and the
  two HWDGE rings' transfers overlap.
- Stage-0 QKV lands in a (32, 288)-allocated PSUM tile; Qk|Kk go
  through ONE psum->sbuf bf16 copy + one DVE 32x32 block transpose
  (garbage rows/cols are never read), V goes straight psum->sb_vm.
- regs and the gamma/beta offset are folded host-side into Wo' and De'
  (De gets a 9th column = offs with gate column 9 = 1.0; Wo' gets a
  97th contraction row = offs with otf row 97 = 1.0), so the FiLM tail
  is 3 DVE ops: d1 = trans''-de'', d2 = d1*alpha, out = d2+de''.
- Stage-2 softmax sum is fused into the Exp ACTIVATE via accum_out;
  scr*v2T + reduce is fused via tensor_tensor_reduce.
- alpha (16 MMs), qT (4), qq2T (1) are emitted into PE queue gaps; the
  PE runs its queue in order, so emission order is placement.
"""

import numpy as np

NM, ZG, HID, SEQ = 512, 512, 96, 8
H1, H2, NPROC, NB = 3, 16, 8, 2
SCL = float(1.0 / np.sqrt(96.0))

# b97 column layout (97 partitions, bf16)
C_F1, C_WQ2, C_WK2, C_WV2, C_F2, C_WO, C_E = 0, 96, 192, 288, 384, 480, 992
B97_COLS = 1008
# pxb (bf16): P chunks (32) | xT (4)
PXB_COLS = 36
# pxf (fp32): baT (4) | De'' (36 = 4 chunks x 9) | gate9 (9) | bqr (1)
F_BA, F_DE, F_G, F_BQR = 0, 4, 40, 49
PXF_COLS = 50

_CACHE = {}


def _build_nc():
    import concourse.bass as bass
    import concourse.bacc as bacc
    import concourse.tile as tile
    import concourse.mybir as mybir

    f32 = mybir.dt.float32
    bf16 = mybir.dt.bfloat16
    AX = mybir.AxisListType
    ALU = mybir.AluOpType
    ACT = mybir.ActivationFunctionType

    nc = bacc.Bacc("TRN2", target_bir_lowering=False, debug=False, num_devices=NPROC)

    d_pxb = nc.dram_tensor("pxb", [128, PXB_COLS], bf16, kind="ExternalInput").ap()
    d_pxf = nc.dram_tensor("pxf", [128, PXF_COLS], f32, kind="ExternalInput").ap()
    d_w1 = nc.dram_tensor("w1", [128, 1152], bf16, kind="ExternalInput").ap()
    d_wqr = nc.dram_tensor("wqr", [128, 384], bf16, kind="ExternalInput").ap()
    d_b97 = nc.dram_tensor("b97", [97, B97_COLS], bf16, kind="ExternalInput").ap()
    d_b16 = nc.dram_tensor("b16", [16, 96], bf16, kind="ExternalInput").ap()
    d_wa = nc.dram_tensor("wa", [128, 2048], bf16, kind="ExternalInput").ap()
    d_out = nc.dram_tensor("out", [128, 4], f32, kind="ExternalOutput").ap()

    with tile.TileContext(nc) as tc, \
         tc.tile_pool(name="sb", bufs=1) as sb, \
         tc.tile_pool(name="ps", bufs=8, space="PSUM") as ps:

        def sbt(shape, tag, dt=f32):
            return sb.tile(shape, dt, tag=tag, name=tag)

        def pst(shape, tag):
            return ps.tile(shape, f32, tag="ps_shared", name=tag)

        # ---- input DMAs: ONE hardware ring (qScalarDynamicHW), consumption
        # order. The sync/gpsimd "rings" share the same 16 DMA engines (so
        # split rings just contend) and qGpSimdDynamic is a slow SOFTWARE
        # DGE. Scalar's first instruction being a waitless dma_start also
        # hoists the framework's ACT_TABLE_LOAD to the queue head where it
        # runs during the transfers instead of stalling the first ACTIVATE.
        sb_pxb = sbt([128, PXB_COLS], "sb_pxb", bf16)
        nc.scalar.dma_start(out=sb_pxb[:], in_=d_pxb[:])
        sb_w1k = []
        for k in range(4):
            t = sbt([128, 288], f"sb_w1k{k}", bf16)
            nc.scalar.dma_start(out=t[:], in_=d_w1[:, 288 * k:288 * k + 288])
            sb_w1k.append(t)
        sb_pxf = sbt([128, PXF_COLS], "sb_pxf")
        nc.scalar.dma_start(out=sb_pxf[:], in_=d_pxf[:])
        sb_wqr = sbt([128, 384], "sb_wqr", bf16)
        nc.scalar.dma_start(out=sb_wqr[:], in_=d_wqr[:])
        sb_97 = sbt([97, B97_COLS], "sb_97", bf16)
        nc.scalar.dma_start(out=sb_97[:], in_=d_b97[:])
        sb_16 = sbt([16, 96], "sb_16", bf16)
        nc.scalar.dma_start(out=sb_16[:], in_=d_b16[:])
        sb_wa = sbt([128, 2048], "sb_wa", bf16)
        nc.scalar.dma_start(out=sb_wa[:], in_=d_wa[:])

        # gpsimd: zero-fills (cheap, off the DMA ring)
        sb_vm = sbt([8, 288], "sb_vm", bf16)
        nc.gpsimd.memset(sb_vm[:], 0.0)
        sb_otf = sbt([97, 1], "sb_otf", bf16)
        nc.gpsimd.memset(sb_otf[96:97, :], 1.0)
        # zero the transpose scratch rows the copies/exp never write (the
        # transposed garbage lands in never-read columns, but the DVE
        # transpose READS the full 32-row blocks)
        sb_t32 = sbt([32, 192], "sb_t32", bf16)
        nc.gpsimd.memset(sb_t32[:], 0.0)
        sb_a32 = sbt([32, 96], "sb_a32", bf16)
        nc.gpsimd.memset(sb_a32[:], 0.0)

        P_blk = lambda k: sb_pxb[:, 8 * k:8 * k + 8]
        xT_blk = lambda k: sb_pxb[:, 32 + k:33 + k]

        # ---- stage 0: [Qk | Kk | Vv] (8, 288) in 4 bf16 matmuls ----
        ps_qkv = pst([32, 288], "ps_qkv")
        for k in range(4):
            nc.tensor.matmul(ps_qkv[0:8, :], P_blk(k), sb_w1k[k][:],
                             start=(k == 0), stop=(k == 3))

        # Qk|Kk -> one bf16 copy into the 32-partition transpose scratch
        # (on DVE: the scalar queue is still draining DMA issues)
        nc.vector.tensor_copy(sb_t32[0:8, :], ps_qkv[0:8, 0:192])
        sb_tT = sbt([32, 192], "sb_tT", bf16)
        nc.vector.transpose(sb_tT[:], sb_t32[:])

        def QkT_h(h):
            return sb_tT[0:32, 32 * h:32 * h + 8]

        def KkT_h(h):
            return sb_tT[0:32, 96 + 32 * h:96 + 32 * h + 8]

        # MHA1 scores, per head
        ps_s = pst([8, 24], "ps_s")
        for h in range(3):
            nc.tensor.matmul(ps_s[:, 8 * h:8 * h + 8], QkT_h(h), KkT_h(h))

        # V blocks straight from PSUM into the 128-col-strided sb_vm with
        # ONE copy (strided dst AP); DVE is idle between transpose and the
        # softmax reduce (gpsimd cannot read PSUM)
        for h in range(3):
            nc.vector.tensor_copy(sb_vm[:, 128 * h:128 * h + 32],
                                  ps_qkv[0:8, 192 + 32 * h:192 + 32 * h + 32])

        # qT: contraction chunks over Wqr (PE gap filler)
        ps_qT = pst([96, 1], "ps_qT")
        for k in range(4):
            nc.tensor.matmul(ps_qT[:], sb_wqr[:, 96 * k:96 * k + 96], xT_blk(k),
                             start=(k == 0), stop=(k == 3))

        # softmax1: exp -> bf16 A, sums, recip (bf16), normalize, transpose
        a32v = sb_a32[0:8, :].rearrange("p (h x) -> p h x", h=3)[:, :, 0:8]
        nc.scalar.activation(a32v, ps_s[:].rearrange("p (h x) -> p h x", h=3),
                             ACT.Exp, scale=SCL)
        sb_sums = sbt([8, 3], "sb_sums")
        nc.vector.tensor_reduce(sb_sums[:], a32v, AX.X, ALU.add)
        sb_rec = sbt([8, 3], "sb_rec", bf16)
        with nc.allow_low_precision(reason="A-normalizer in bf16; output gate 2e-2"):
            nc.vector.reciprocal(sb_rec[:], sb_sums[:])
        rec_ap = sb_rec[:]
        rec_bc = bass.AP(tensor=rec_ap.tensor, offset=rec_ap.offset,
                         ap=[rec_ap.ap[0], rec_ap.ap[1], [0, 8]])
        nc.vector.tensor_tensor(a32v, a32v, rec_bc, ALU.mult)
        sb_aT32 = sbt([32, 96], "sb_aT32", bf16)
        nc.vector.transpose(sb_aT32[:], sb_a32[:])

        def A_T(h):
            return sb_aT32[0:8, 32 * h:32 * h + 8]

        # QkT (96,8) fp32 for the attention residual (off critical path;
        # gpsimd can copy SBUF->SBUF)
        sb_qkT = sbt([96, 8], "sb_qkT")
        for h in range(3):
            nc.gpsimd.tensor_copy(sb_qkT[32 * h:32 * h + 32, :], QkT_h(h))
        # q relu (bias bqr) -> bf16
        sb_qTb = sbt([96, 1], "sb_qTb", bf16)
        nc.scalar.activation(sb_qTb[:], ps_qT[:], ACT.Relu,
                             bias=sb_pxf[0:96, F_BQR:F_BQR + 1])
        sb_qqT = sbt([96, 1], "sb_qqT")

        # O^T = sum_h V_h^T-layout @ A_T_h (psum-offset accumulate trick)
        ps_oT = pst([96, 8], "ps_oT")
        for h in range(3):
            nc.tensor.matmul(ps_oT[:], sb_vm[:, 96 * h:96 * h + 96], A_T(h),
                             start=(h == 0), stop=(h == 2))
        # qq2T gap filler (needs b97 + qTb only)
        ps_qqT = pst([96, 1], "ps_qqT")
        nc.tensor.matmul(ps_qqT[:], sb_97[0:96, C_WQ2:C_WQ2 + 96], sb_qTb[:])

        sb_hT = sbt([96, 8], "sb_hT", bf16)
        nc.vector.tensor_add(sb_hT[:], ps_oT[:], sb_qkT[:])

        # ---- fc1 residual ----
        ps_rT = pst([96, 8], "ps_rT")
        nc.tensor.matmul(ps_rT[:], sb_97[0:96, C_F1:C_F1 + 96], sb_hT[:])
        nc.scalar.copy(sb_qqT[:], ps_qqT[:])
        sb_rT = sbt([96, 8], "sb_rT", bf16)
        nc.scalar.activation(sb_rT[:], ps_rT[:], ACT.Relu)
        sb_h2T = sbt([96, 8], "sb_h2T", bf16)
        nc.vector.tensor_add(sb_h2T[:], sb_hT[:], sb_rT[:])

        # de on gpsimd: (128,4,9) * gate9 -> reduce (9th col carries offs)
        sb_de = sbt([128, 4], "sb_de")
        sb_dp = sbt([128, 36], "sb_dp")
        de_v = sb_pxf[:, F_DE:F_DE + 36].rearrange("p (m s) -> p m s", m=4)
        g_ap = sb_pxf[:, F_G:F_G + 9]
        g_bc = bass.AP(tensor=g_ap.tensor, offset=g_ap.offset,
                       ap=[g_ap.ap[0], [0, 4], g_ap.ap[1]])
        nc.gpsimd.tensor_tensor(sb_dp[:].rearrange("p (m s) -> p m s", m=4),
                                de_v, g_bc, ALU.mult)
        nc.vector.tensor_reduce(sb_de[:],
                                sb_dp[:].rearrange("p (m s) -> p m s", m=4),
                                AX.X, ALU.add)

        # ---- stage 2 ----
        ps_k2T = pst([96, 8], "ps_k2T")
        nc.tensor.matmul(ps_k2T[:], sb_97[0:96, C_WK2:C_WK2 + 96], sb_h2T[:])
        ps_v2T = pst([96, 8], "ps_v2T")
        nc.tensor.matmul(ps_v2T[:], sb_97[0:96, C_WV2:C_WV2 + 96], sb_h2T[:])

        qq_ap = sb_qqT[:]
        qq_bc = bass.AP(tensor=qq_ap.tensor, offset=qq_ap.offset,
                        ap=[qq_ap.ap[0], [0, 8]])
        sb_tmp = sbt([96, 8], "sb_tmp", bf16)
        nc.vector.tensor_tensor(sb_tmp[:], ps_k2T[:], qq_bc, ALU.mult)
        sb_v2T = sbt([96, 8], "sb_v2T")
        nc.scalar.copy(sb_v2T[:], ps_v2T[:])

        ps_s2 = pst([16, 8], "ps_s2")
        nc.tensor.matmul(ps_s2[:], sb_97[0:96, C_E:C_E + 16], sb_tmp[:])

        # alpha: 16 (k,m) chunk matmuls, emitted into the PE gaps opened
        # by the stage-2 scalar/vector chain (wa has landed by now)
        ps_al = pst([128, 4], "ps_al")

        def alpha_mms(ms):
            for m in ms:
                for k in range(4):
                    nc.tensor.matmul(
                        ps_al[:, m:m + 1],
                        sb_wa[:, 512 * k + 128 * m:512 * k + 128 * m + 128],
                        xT_blk(k), start=(k == 0), stop=(k == 3))

        alpha_mms([0, 1])

        sb_e2 = sbt([16, 8], "sb_e2")
        sb_sum2 = sbt([16, 1], "sb_sum2")
        nc.scalar.activation(sb_e2[:], ps_s2[:], ACT.Exp, scale=SCL)
        nc.vector.tensor_reduce(sb_sum2[:], sb_e2[:], AX.X, ALU.add)
        sb_rec2 = sbt([16, 1], "sb_rec2")
        nc.vector.reciprocal(sb_rec2[:], sb_sum2[:])
        r2_ap = sb_rec2[:]
        r2_bc = bass.AP(tensor=r2_ap.tensor, offset=r2_ap.offset,
                        ap=[r2_ap.ap[0], [0, 8]])
        sb_a2 = sbt([16, 8], "sb_a2", bf16)
        nc.vector.tensor_tensor(sb_a2[:], sb_e2[:], r2_bc, ALU.mult)

        ps_a2e = pst([96, 8], "ps_a2e")
        nc.tensor.matmul(ps_a2e[:], sb_16[:], sb_a2[:])
        alpha_mms([2])
        sb_scr = sbt([96, 8], "sb_scr")
        sb_o2T = sbt([96, 1], "sb_o2T")
        nc.vector.tensor_mul(sb_scr[:], ps_a2e[:], sb_v2T[:])
        nc.vector.tensor_reduce(sb_o2T[:], sb_scr[:], AX.X, ALU.add)
        sb_ot1 = sbt([96, 1], "sb_ot1", bf16)
        nc.vector.tensor_add(sb_ot1[:], sb_o2T[:], sb_qqT[:])

        ps_r2 = pst([96, 1], "ps_r2")
        nc.tensor.matmul(ps_r2[:], sb_97[0:96, C_F2:C_F2 + 96], sb_ot1[:])
        alpha_mms([3])
        sb_r2 = sbt([96, 1], "sb_r2", bf16)
        nc.scalar.activation(sb_r2[:], ps_r2[:], ACT.Relu)
        nc.vector.tensor_add(sb_otf[0:96, :], sb_ot1[:], sb_r2[:])

        # alpha sigmoid tail: 1/(1+exp(-(z+ba))), off critical path
        # (gpsimd cannot read PSUM, so zb is on DVE)
        sb_zb = sbt([128, 4], "sb_zb")
        nc.vector.tensor_add(sb_zb[:], ps_al[:], sb_pxf[:, F_BA:F_BA + 4])
        sb_en = sbt([128, 4], "sb_en")
        nc.scalar.activation(sb_en[:], sb_zb[:], ACT.Exp, scale=-1.0)
        sb_dn = sbt([128, 4], "sb_dn")
        nc.gpsimd.tensor_scalar_add(sb_dn[:], sb_en[:], 1.0)
        sb_alp = sbt([128, 4], "sb_alp")
        nc.vector.reciprocal(sb_alp[:], sb_dn[:])

        # trans'' = otf97 @ Wo' (regs and offs folded host-side)
        ps_tr = pst([128, 4], "ps_tr")
        for m in range(4):
            nc.tensor.matmul(ps_tr[:, m:m + 1],
                             sb_97[0:97, C_WO + 128 * m:C_WO + 128 * m + 128],
                             sb_otf[:])

        # FiLM tail: 3 DVE ops
        sb_d1 = sbt([128, 4], "sb_d1")
        nc.vector.tensor_sub(sb_d1[:], ps_tr[:], sb_de[:])
        sb_d2 = sbt([128, 4], "sb_d2")
        nc.vector.tensor_mul(sb_d2[:], sb_d1[:], sb_alp[:])
        sb_o = sbt([128, 4], "sb_o")
        nc.vector.tensor_add(sb_o[:], sb_d2[:], sb_de[:])

        nc.scalar.dma_start(out=d_out[:], in_=sb_o[:])

    nc.compile()
    return nc


def _to_chunks128(a, cols):
    """(512, cols) -> (128, 4*cols) with column block k = rows [128k, 128k+128)."""
    return np.ascontiguousarray(
        a.reshape(4, 128, cols).transpose(1, 0, 2).reshape(128, 4 * cols),
        dtype=np.float32)


def _pack_inputs(inputs):
    import ml_dtypes
    bf = ml_dtypes.bfloat16

    gate = np.asarray(inputs['gate'], np.float32)
    x = np.asarray(inputs['x'], np.float32)
    Wa = np.asarray(inputs['Wa'], np.float32)
    ba = np.asarray(inputs['ba'], np.float32)
    Wqr = np.asarray(inputs['Wqr'], np.float32)
    bqr = np.asarray(inputs['bqr'], np.float32)
    P = np.asarray(inputs['P'], np.float32)
    De = np.asarray(inputs['De'], np.float32)
    regs = np.asarray(inputs['regs'], np.float32)

    wa_p = np.ascontiguousarray(_to_chunks128(Wa, 512).astype(bf))
    wqr_p = np.ascontiguousarray(_to_chunks128(Wqr, 96).astype(bf))
    xT4 = np.ascontiguousarray(x.reshape(4, 128).T, np.float32)
    baT4 = np.ascontiguousarray(ba.reshape(4, 128).T, np.float32)
    g9 = np.concatenate([gate.reshape(1, 8), [[1.0]]], axis=1).astype(np.float32)
    g128 = np.ascontiguousarray(np.tile(g9, (128, 1)))
    bqr_col = np.zeros((128, 1), np.float32)
    bqr_col[0:96, 0] = bqr

    E = np.zeros((96, 16), np.float32)
    E[np.arange(96), np.arange(96) // 6] = 1.0
    b16 = np.ascontiguousarray(E.T.astype(bf))

    in_maps = []
    for i in range(NPROC):
        b, t = i // 4, i % 4
        offs = 1.0 if t in (0, 2) else 0.0
        rg = regs[b, t]                                     # (512,)
        # De'' = De*regs with a 9th column = offs (gate9[8] = 1.0)
        De9 = np.concatenate([De[b, t] * rg[:, None],
                              np.full((NM, 1), offs, np.float32)], axis=1)
        pxb = np.concatenate([_to_chunks128(P[b, t], 8), xT4], axis=1)
        pxf = np.concatenate([baT4, _to_chunks128(De9, 9), g128, bqr_col],
                             axis=1)
        wq1 = np.asarray(inputs['Wq1'], np.float32)[i]
        wk1 = np.asarray(inputs['Wk1'], np.float32)[i]
        wv1 = np.asarray(inputs['Wv1'], np.float32)[i]
        w1 = np.concatenate(
            [np.concatenate([wq1[128 * k:128 * k + 128],
                             wk1[128 * k:128 * k + 128],
                             wv1[128 * k:128 * k + 128]], axis=1)
             for k in range(4)], axis=1)
        b97 = np.zeros((97, B97_COLS), np.float32)
        b97[0:96, C_F1:C_F1 + 96] = np.asarray(inputs['fc1'], np.float32)[i]
        b97[0:96, C_WQ2:C_WQ2 + 96] = np.asarray(inputs['Wq2'], np.float32)[i]
        b97[0:96, C_WK2:C_WK2 + 96] = np.asarray(inputs['Wk2'], np.float32)[i]
        b97[0:96, C_WV2:C_WV2 + 96] = np.asarray(inputs['Wv2'], np.float32)[i]
        b97[0:96, C_F2:C_F2 + 96] = np.asarray(inputs['fc2'], np.float32)[i]
        b97[0:96, C_WO:C_WO + 512] = (np.asarray(inputs['Wo'], np.float32)[i]
                                      * rg[None, :])
        b97[96, C_WO:C_WO + 512] = offs
        b97[0:96, C_E:C_E + 16] = E
        in_maps.append({
            'pxb': np.ascontiguousarray(pxb.astype(bf)),
            'pxf': np.ascontiguousarray(pxf),
            'w1': np.ascontiguousarray(w1.astype(bf)),
            'wqr': wqr_p,
            'b97': np.ascontiguousarray(b97.astype(bf)),
            'b16': b16,
            'wa': wa_p,
        })
    return in_maps


def _run(inputs, trace=False):
    from concourse.bass_utils import run_bass_kernel_spmd
    if 'nc' not in _CACHE:
        _CACHE['nc'] = _build_nc()
    nc = _CACHE['nc']
    in_maps = _pack_inputs(inputs)
    res = run_bass_kernel_spmd(nc, in_maps, list(range(NPROC)), trace=trace)
    out = np.zeros((NB, 4, NM), np.float32)
    for i in range(NPROC):
        out[i // 4, i % 4] = np.asarray(res.results[i]['out']).T.reshape(NM)
    return out, res


def kernel(**inputs):
    out, _ = _run(inputs, trace=False)
    return out


# revision 20
# speedup vs baseline: 1.3489x; 1.3489x over previous
"""Trainium2 Bass kernel for nn_FilmLayerNetwork.

Sharding: one NeuronCore per SMAB processor (NPROC = 8 = n_cores).
Each core computes its processor's full 512-map output slice.

v4 performance notes (v1 notes in kernel_v1.py.bak):
- ALL matmuls bf16 (fp32 MMs are LOW/HIGH pairs; error budget is huge:
  trans feeds mix*regs with regs ~1e-3, gate 2e-2). alpha's Wa is fp8
  (e4m3) - it only feeds a sigmoid whose output scales a ~1e-4 term.
- DMA is PACKET-RATE bound (~80ns per dst-partition row per engine) and
  rows stripe across 16 engines ONLY for 128-partition transfers: a
  97-row transfer lands entirely on ONE engine (measured: 97x2016B on
  engine 64 at ~810ns/row = 9us stall). So: b97 is DMA'd as a 96-row +
  a 1-row transfer, E^T is folded into b97 columns (kills the 16-row
  b16 DMA), and pxb+w1k0 / w1k1-3 are merged into two wide-row
  transfers. All input DMAs go on the ONE hardware ring
  (qScalarDynamicHW) in consumption order - split engine-rings share
  the same 16 DMA engines and just contend, and qGpSimdDynamic is a
  slow software DGE.
- Scalar's first instruction is a waitless dma_start so the framework
  hoists ACT_TABLE_LOAD to the queue head (it otherwise runs right
  before the first ACTIVATE, costing 1.3us on the critical path).
- Stage-0 emits the Qk|Kk column group's 4 accumulation MMs before the
  Vv group so the DVE transpose starts as soon as QK is done.
- regs and the gamma/beta offset are folded host-side into Wo' and De'
  (9th De column = offs with gate9[8]=1.0; 97th Wo' row = offs with
  otf[96]=1.0), so the FiLM tail is 3 DVE ops.
- alpha (16 MMs), qT (4), qq2T (1) fill PE gaps; copies ride on
  vector/gpsimd so the scalar queue stays clear for the softmax exps.
"""

import numpy as np

NM, ZG, HID, SEQ = 512, 512, 96, 8
H1, H2, NPROC, NB = 3, 16, 8, 2
SCL = float(1.0 / np.sqrt(96.0))

# b97 column layout (97 partitions, bf16); E^T lives in rows 0:16 of C_ET
C_F1, C_WQ2, C_WK2, C_WV2, C_F2, C_WO, C_E, C_ET = (
    0, 96, 192, 288, 384, 480, 992, 1008)
B97_COLS = 1104
# w0 (bf16): P chunks (32) | xT (4) | w1 chunk0 (288)
W0_COLS = 324
# pxf (fp32): baT (4) | De'' (36 = 4 chunks x 9) | gate9 (9) | bqr (1)
F_BA, F_DE, F_G, F_BQR = 0, 4, 40, 49
PXF_COLS = 50

_CACHE = {}


def _build_nc():
    import concourse.bass as bass
    import concourse.bacc as bacc
    import concourse.tile as tile
    import concourse.mybir as mybir

    f32 = mybir.dt.float32
    bf16 = mybir.dt.bfloat16
    fp8 = mybir.dt.float8e4
    AX = mybir.AxisListType
    ALU = mybir.AluOpType
    ACT = mybir.ActivationFunctionType

    nc = bacc.Bacc("TRN2", target_bir_lowering=False, debug=False, num_devices=NPROC)

    d_w0 = nc.dram_tensor("w0", [128, W0_COLS], bf16, kind="ExternalInput").ap()
    d_w1r = nc.dram_tensor("w1r", [128, 864], bf16, kind="ExternalInput").ap()
    d_pxf = nc.dram_tensor("pxf", [128, PXF_COLS], f32, kind="ExternalInput").ap()
    d_wqr = nc.dram_tensor("wqr", [128, 384], bf16, kind="ExternalInput").ap()
    d_b97 = nc.dram_tensor("b97", [97, B97_COLS], bf16, kind="ExternalInput").ap()
    d_wa = nc.dram_tensor("wa", [128, 2048], fp8, kind="ExternalInput").ap()
    d_out = nc.dram_tensor("out", [128, 4], f32, kind="ExternalOutput").ap()

    with tile.TileContext(nc) as tc, \
         tc.tile_pool(name="sb", bufs=1) as sb, \
         tc.tile_pool(name="ps", bufs=8, space="PSUM") as ps:

        def sbt(shape, tag, dt=f32):
            return sb.tile(shape, dt, tag=tag, name=tag)

        def pst(shape, tag):
            return ps.tile(shape, f32, tag="ps_shared", name=tag)

        # ---- input DMAs: one hardware ring, consumption order ----
        sb_w0 = sbt([128, W0_COLS], "sb_w0", bf16)
        nc.scalar.dma_start(out=sb_w0[:], in_=d_w0[:])
        sb_w1r = sbt([128, 864], "sb_w1r", bf16)
        nc.scalar.dma_start(out=sb_w1r[:], in_=d_w1r[:])
        sb_pxf = sbt([128, PXF_COLS], "sb_pxf")
        nc.scalar.dma_start(out=sb_pxf[:], in_=d_pxf[:])
        sb_wqr = sbt([128, 384], "sb_wqr", bf16)
        nc.scalar.dma_start(out=sb_wqr[:], in_=d_wqr[:])
        sb_97 = sbt([97, B97_COLS], "sb_97", bf16)
        nc.scalar.dma_start(out=sb_97[0:96, :], in_=d_b97[0:96, :])
        nc.scalar.dma_start(out=sb_97[96:97, :], in_=d_b97[96:97, :])
        sb_wa = sbt([128, 2048], "sb_wa", fp8)
        nc.scalar.dma_start(out=sb_wa[:], in_=d_wa[:])

        # gpsimd: zero-fills + fp8 x for the alpha matmuls
        sb_vm = sbt([8, 288], "sb_vm", bf16)
        nc.gpsimd.memset(sb_vm[:], 0.0)
        sb_otf = sbt([97, 1], "sb_otf", bf16)
        nc.gpsimd.memset(sb_otf[96:97, :], 1.0)
        sb_t32 = sbt([32, 192], "sb_t32", bf16)
        nc.gpsimd.memset(sb_t32[:], 0.0)
        sb_a32 = sbt([32, 96], "sb_a32", bf16)
        nc.gpsimd.memset(sb_a32[:], 0.0)

        P_blk = lambda k: sb_w0[:, 8 * k:8 * k + 8]
        xT_blk = lambda k: sb_w0[:, 32 + k:33 + k]
        w1_blk = lambda k: (sb_w0[:, 36:324] if k == 0
                            else sb_w1r[:, 288 * (k - 1):288 * (k - 1) + 288])
        sb_x8 = sbt([128, 4], "sb_x8", fp8)
        nc.gpsimd.tensor_copy(sb_x8[:], sb_w0[:, 32:36])

        # ---- stage 0: [Qk|Kk] then [Vv], chunk-gated accumulations ----
        ps_qkv = pst([32, 288], "ps_qkv")
        for k in range(4):
            nc.tensor.matmul(ps_qkv[0:8, 0:192], P_blk(k), w1_blk(k)[:, 0:192],
                             start=(k == 0), stop=(k == 3))
        for k in range(4):
            nc.tensor.matmul(ps_qkv[0:8, 192:288], P_blk(k), w1_blk(k)[:, 192:288],
                             start=(k == 0), stop=(k == 3))

        # Qk|Kk -> one bf16 copy into the 32-partition transpose scratch
        # (on DVE: the scalar queue is still draining DMA issues)
        nc.vector.tensor_copy(sb_t32[0:8, :], ps_qkv[0:8, 0:192])
        sb_tT = sbt([32, 192], "sb_tT", bf16)
        nc.vector.transpose(sb_tT[:], sb_t32[:])

        def QkT_h(h):
            return sb_tT[0:32, 32 * h:32 * h + 8]

        def KkT_h(h):
            return sb_tT[0:32, 96 + 32 * h:96 + 32 * h + 8]

        # MHA1 scores, per head
        ps_s = pst([8, 24], "ps_s")
        for h in range(3):
            nc.tensor.matmul(ps_s[:, 8 * h:8 * h + 8], QkT_h(h), KkT_h(h))

        # qT: contraction chunks over Wqr (PE gap filler)
        ps_qT = pst([96, 1], "ps_qT")
        for k in range(4):
            nc.tensor.matmul(ps_qT[:], sb_wqr[:, 96 * k:96 * k + 96], xT_blk(k),
                             start=(k == 0), stop=(k == 3))

        # softmax1: exp -> bf16 A, sums, recip (bf16), normalize, transpose
        a32v = sb_a32[0:8, :].rearrange("p (h x) -> p h x", h=3)[:, :, 0:8]
        nc.scalar.activation(a32v, ps_s[:].rearrange("p (h x) -> p h x", h=3),
                             ACT.Exp, scale=SCL)
        sb_sums = sbt([8, 3], "sb_sums")
        nc.vector.tensor_reduce(sb_sums[:], a32v, AX.X, ALU.add)
        sb_rec = sbt([8, 3], "sb_rec", bf16)
        with nc.allow_low_precision(reason="A-normalizer in bf16; output gate 2e-2"):
            nc.vector.reciprocal(sb_rec[:], sb_sums[:])
        rec_ap = sb_rec[:]
        rec_bc = bass.AP(tensor=rec_ap.tensor, offset=rec_ap.offset,
                         ap=[rec_ap.ap[0], rec_ap.ap[1], [0, 8]])
        nc.vector.tensor_tensor(a32v, a32v, rec_bc, ALU.mult)

        # V blocks straight from PSUM into the 128-col-strided sb_vm with
        # one strided-dst DVE copy
        vm_ap = sb_vm[:]
        vm_dst = bass.AP(tensor=vm_ap.tensor, offset=vm_ap.offset,
                         ap=[vm_ap.ap[0], [128, 3], [1, 32]])
        nc.vector.tensor_copy(
            vm_dst, ps_qkv[0:8, 192:288].rearrange("p (h c) -> p h c", h=3))

        sb_aT32 = sbt([32, 96], "sb_aT32", bf16)
        nc.vector.transpose(sb_aT32[:], sb_a32[:])

        def A_T(h):
            return sb_aT32[0:8, 32 * h:32 * h + 8]

        # QkT (96,8) fp32 for the attention residual (gpsimd, SBUF->SBUF)
        sb_qkT = sbt([96, 8], "sb_qkT")
        for h in range(3):
            nc.gpsimd.tensor_copy(sb_qkT[32 * h:32 * h + 32, :], QkT_h(h))
        # q relu (bias bqr) -> bf16
        sb_qTb = sbt([96, 1], "sb_qTb", bf16)
        nc.scalar.activation(sb_qTb[:], ps_qT[:], ACT.Relu,
                             bias=sb_pxf[0:96, F_BQR:F_BQR + 1])
        sb_qqT = sbt([96, 1], "sb_qqT")

        # O^T = sum_h V_h^T-layout @ A_T_h (psum-offset accumulate trick)
        ps_oT = pst([96, 8], "ps_oT")
        for h in range(3):
            nc.tensor.matmul(ps_oT[:], sb_vm[:, 96 * h:96 * h + 96], A_T(h),
                             start=(h == 0), stop=(h == 2))
        # qq2T gap filler (needs b97 + qTb only)
        ps_qqT = pst([96, 1], "ps_qqT")
        nc.tensor.matmul(ps_qqT[:], sb_97[0:96, C_WQ2:C_WQ2 + 96], sb_qTb[:])

        sb_hT = sbt([96, 8], "sb_hT", bf16)
        nc.vector.tensor_add(sb_hT[:], ps_oT[:], sb_qkT[:])

        # ---- fc1 residual ----
        ps_rT = pst([96, 8], "ps_rT")
        nc.tensor.matmul(ps_rT[:], sb_97[0:96, C_F1:C_F1 + 96], sb_hT[:])
        nc.scalar.copy(sb_qqT[:], ps_qqT[:])
        sb_rT = sbt([96, 8], "sb_rT", bf16)
        nc.scalar.activation(sb_rT[:], ps_rT[:], ACT.Relu)
        sb_h2T = sbt([96, 8], "sb_h2T", bf16)
        nc.vector.tensor_add(sb_h2T[:], sb_hT[:], sb_rT[:])

        # de on gpsimd/vector: (128,4,9) * gate9 -> reduce (9th col = offs)
        sb_de = sbt([128, 4], "sb_de")
        sb_dp = sbt([128, 36], "sb_dp")
        de_v = sb_pxf[:, F_DE:F_DE + 36].rearrange("p (m s) -> p m s", m=4)
        g_ap = sb_pxf[:, F_G:F_G + 9]
        g_bc = bass.AP(tensor=g_ap.tensor, offset=g_ap.offset,
                       ap=[g_ap.ap[0], [0, 4], g_ap.ap[1]])
        nc.gpsimd.tensor_tensor(sb_dp[:].rearrange("p (m s) -> p m s", m=4),
                                de_v, g_bc, ALU.mult)
        nc.vector.tensor_reduce(sb_de[:],
                                sb_dp[:].rearrange("p (m s) -> p m s", m=4),
                                AX.X, ALU.add)

        # ---- stage 2 ----
        ps_k2T = pst([96, 8], "ps_k2T")
        nc.tensor.matmul(ps_k2T[:], sb_97[0:96, C_WK2:C_WK2 + 96], sb_h2T[:])
        ps_v2T = pst([96, 8], "ps_v2T")
        nc.tensor.matmul(ps_v2T[:], sb_97[0:96, C_WV2:C_WV2 + 96], sb_h2T[:])

        qq_ap = sb_qqT[:]
        qq_bc = bass.AP(tensor=qq_ap.tensor, offset=qq_ap.offset,
                        ap=[qq_ap.ap[0], [0, 8]])
        sb_tmp = sbt([96, 8], "sb_tmp", bf16)
        nc.vector.tensor_tensor(sb_tmp[:], ps_k2T[:], qq_bc, ALU.mult)
        sb_v2T = sbt([96, 8], "sb_v2T")
        nc.scalar.copy(sb_v2T[:], ps_v2T[:])

        ps_s2 = pst([16, 8], "ps_s2")
        nc.tensor.matmul(ps_s2[:], sb_97[0:96, C_E:C_E + 16], sb_tmp[:])

        # alpha: 16 (k,m) fp8 chunk matmuls in the stage-2 PE gaps
        ps_al = pst([128, 4], "ps_al")

        def alpha_mms(ms):
            for m in ms:
                for k in range(4):
                    nc.tensor.matmul(
                        ps_al[:, m:m + 1],
                        sb_wa[:, 512 * k + 128 * m:512 * k + 128 * m + 128],
                        sb_x8[:, k:k + 1], start=(k == 0), stop=(k == 3))

        alpha_mms([0, 1])

        sb_e2 = sbt([16, 8], "sb_e2")
        sb_sum2 = sbt([16, 1], "sb_sum2")
        nc.scalar.activation(sb_e2[:], ps_s2[:], ACT.Exp, scale=SCL)
        nc.vector.tensor_reduce(sb_sum2[:], sb_e2[:], AX.X, ALU.add)
        sb_rec2 = sbt([16, 1], "sb_rec2")
        nc.vector.reciprocal(sb_rec2[:], sb_sum2[:])
        r2_ap = sb_rec2[:]
        r2_bc = bass.AP(tensor=r2_ap.tensor, offset=r2_ap.offset,
                        ap=[r2_ap.ap[0], [0, 8]])
        sb_a2 = sbt([16, 8], "sb_a2", bf16)
        nc.vector.tensor_tensor(sb_a2[:], sb_e2[:], r2_bc, ALU.mult)

        ps_a2e = pst([96, 8], "ps_a2e")
        nc.tensor.matmul(ps_a2e[:], sb_97[0:16, C_ET:C_ET + 96], sb_a2[:])
        alpha_mms([2])
        sb_scr = sbt([96, 8], "sb_scr")
        sb_o2T = sbt([96, 1], "sb_o2T")
        nc.vector.tensor_mul(sb_scr[:], ps_a2e[:], sb_v2T[:])
        nc.vector.tensor_reduce(sb_o2T[:], sb_scr[:], AX.X, ALU.add)
        sb_ot1 = sbt([96, 1], "sb_ot1", bf16)
        nc.vector.tensor_add(sb_ot1[:], sb_o2T[:], sb_qqT[:])

        ps_r2 = pst([96, 1], "ps_r2")
        nc.tensor.matmul(ps_r2[:], sb_97[0:96, C_F2:C_F2 + 96], sb_ot1[:])
        alpha_mms([3])
        sb_r2 = sbt([96, 1], "sb_r2", bf16)
        nc.scalar.activation(sb_r2[:], ps_r2[:], ACT.Relu)
        nc.vector.tensor_add(sb_otf[0:96, :], sb_ot1[:], sb_r2[:])

        # alpha sigmoid tail: 1/(1+exp(-(z+ba))), off critical path
        sb_zb = sbt([128, 4], "sb_zb")
        nc.vector.tensor_add(sb_zb[:], ps_al[:], sb_pxf[:, F_BA:F_BA + 4])
        sb_en = sbt([128, 4], "sb_en")
        nc.scalar.activation(sb_en[:], sb_zb[:], ACT.Exp, scale=-1.0)
        sb_dn = sbt([128, 4], "sb_dn")
        nc.gpsimd.tensor_scalar_add(sb_dn[:], sb_en[:], 1.0)
        sb_alp = sbt([128, 4], "sb_alp")
        nc.vector.reciprocal(sb_alp[:], sb_dn[:])

        # trans'' = otf97 @ Wo' (regs and offs folded host-side)
        ps_tr = pst([128, 4], "ps_tr")
        for m in range(4):
            nc.tensor.matmul(ps_tr[:, m:m + 1],
                             sb_97[0:97, C_WO + 128 * m:C_WO + 128 * m + 128],
                             sb_otf[:])

        # FiLM tail: 3 DVE ops
        sb_d1 = sbt([128, 4], "sb_d1")
        nc.vector.tensor_sub(sb_d1[:], ps_tr[:], sb_de[:])
        sb_d2 = sbt([128, 4], "sb_d2")
        nc.vector.tensor_mul(sb_d2[:], sb_d1[:], sb_alp[:])
        sb_o = sbt([128, 4], "sb_o")
        nc.vector.tensor_add(sb_o[:], sb_d2[:], sb_de[:])

        nc.scalar.dma_start(out=d_out[:], in_=sb_o[:])

    nc.compile()
    return nc


def _to_chunks128(a, cols):
    """(512, cols) -> (128, 4*cols) with column block k = rows [128k, 128k+128)."""
    return np.ascontiguousarray(
        a.reshape(4, 128, cols).transpose(1, 0, 2).reshape(128, 4 * cols),
        dtype=np.float32)


def _pack_inputs(inputs):
    import ml_dtypes
    bf = ml_dtypes.bfloat16
    f8 = ml_dtypes.float8_e4m3fn

    gate = np.asarray(inputs['gate'], np.float32)
    x = np.asarray(inputs['x'], np.float32)
    Wa = np.asarray(inputs['Wa'], np.float32)
    ba = np.asarray(inputs['ba'], np.float32)
    Wqr = np.asarray(inputs['Wqr'], np.float32)
    bqr = np.asarray(inputs['bqr'], np.float32)
    P = np.asarray(inputs['P'], np.float32)
    De = np.asarray(inputs['De'], np.float32)
    regs = np.asarray(inputs['regs'], np.float32)

    wa_p = np.ascontiguousarray(_to_chunks128(Wa, 512).astype(f8))
    wqr_p = np.ascontiguousarray(_to_chunks128(Wqr, 96).astype(bf))
    xT4 = np.ascontiguousarray(x.reshape(4, 128).T, np.float32)
    baT4 = np.ascontiguousarray(ba.reshape(4, 128).T, np.float32)
    g9 = np.concatenate([gate.reshape(1, 8), [[1.0]]], axis=1).astype(np.float32)
    g128 = np.ascontiguousarray(np.tile(g9, (128, 1)))
    bqr_col = np.zeros((128, 1), np.float32)
    bqr_col[0:96, 0] = bqr

    E = np.zeros((96, 16), np.float32)
    E[np.arange(96), np.arange(96) // 6] = 1.0

    in_maps = []
    for i in range(NPROC):
        b, t = i // 4, i % 4
        offs = 1.0 if t in (0, 2) else 0.0
        rg = regs[b, t]                                     # (512,)
        De9 = np.concatenate([De[b, t] * rg[:, None],
                              np.full((NM, 1), offs, np.float32)], axis=1)
        pxf = np.concatenate([baT4, _to_chunks128(De9, 9), g128, bqr_col],
                             axis=1)
        wq1 = np.asarray(inputs['Wq1'], np.float32)[i]
        wk1 = np.asarray(inputs['Wk1'], np.float32)[i]
        wv1 = np.asarray(inputs['Wv1'], np.float32)[i]
        w1c = [np.concatenate([wq1[128 * k:128 * k + 128],
                               wk1[128 * k:128 * k + 128],
                               wv1[128 * k:128 * k + 128]], axis=1)
               for k in range(4)]
        w0 = np.concatenate([_to_chunks128(P[b, t], 8), xT4, w1c[0]], axis=1)
        w1r = np.concatenate(w1c[1:], axis=1)
        b97 = np.zeros((97, B97_COLS), np.float32)
        b97[0:96, C_F1:C_F1 + 96] = np.asarray(inputs['fc1'], np.float32)[i]
        b97[0:96, C_WQ2:C_WQ2 + 96] = np.asarray(inputs['Wq2'], np.float32)[i]
        b97[0:96, C_WK2:C_WK2 + 96] = np.asarray(inputs['Wk2'], np.float32)[i]
        b97[0:96, C_WV2:C_WV2 + 96] = np.asarray(inputs['Wv2'], np.float32)[i]
        b97[0:96, C_F2:C_F2 + 96] = np.asarray(inputs['fc2'], np.float32)[i]
        b97[0:96, C_WO:C_WO + 512] = (np.asarray(inputs['Wo'], np.float32)[i]
                                      * rg[None, :])
        b97[96, C_WO:C_WO + 512] = offs
        b97[0:96, C_E:C_E + 16] = E
        b97[0:16, C_ET:C_ET + 96] = E.T
        in_maps.append({
            'w0': np.ascontiguousarray(w0.astype(bf)),
            'w1r': np.ascontiguousarray(w1r.astype(bf)),
            'pxf': np.ascontiguousarray(pxf),
            'wqr': wqr_p,
            'b97': np.ascontiguousarray(b97.astype(bf)),
            'wa': wa_p,
        })
    return in_maps


def _run(inputs, trace=False):
    from concourse.bass_utils import run_bass_kernel_spmd
    if 'nc' not in _CACHE:
        _CACHE['nc'] = _build_nc()
    nc = _CACHE['nc']
    in_maps = _pack_inputs(inputs)
    res = run_bass_kernel_spmd(nc, in_maps, list(range(NPROC)), trace=trace)
    out = np.zeros((NB, 4, NM), np.float32)
    for i in range(NPROC):
        out[i // 4, i % 4] = np.asarray(res.results[i]['out']).T.reshape(NM)
    return out, res


def kernel(**inputs):
    out, _ = _run(inputs, trace=False)
    return out
